# revision 132
# baseline (speedup 1.0000x reference)
"""Trainium2 Bass kernel for LoopRelationalGraphConvolution.

Math (matches the jax reference):
    out[n] = relu( SCALE * sum_s  W[rel[n,s]] @ emb[neighbors[n,s]] )
    SCALE  = 1000 / (R1 * S)      (folds the mean over S and the /R1 * 1000)

Design (8 NeuronCores, data-parallel over the 8192-node batch):
  A two-level host balancer assigns nodes to cores and then to 8 tiles of
  exactly 128 nodes per core such that every (tile, relation) bucket has
  <=128 edges (zero overflow).  Per tile the device kernel:
    1. dma_gather(transpose=True): fetches the tile's 33*128 edge-slot
       embeddings from a combined fp8 table whose 512-byte rows interleave
       e8 = fp8(emb) and er8 = fp8(emb - e8) per dimension, so the 16-bit
       transpose granularity lands  ET[p, c, i, b] = (e8|er8)[idx_i][c*128+p].
       The first NDENSE tiles instead load host-pre-gathered dense images
       with plain DMAs, eliminating the idx->prep->trigger latency chain at
       pipeline startup (which is otherwise serialized behind the W load).
    2. stage-1: per relation r, THREE fp8 DoubleRow matmuls accumulate
       Y[slot, o] = e8*w8 + er8*w8 + e8*wr8   (PSUM f32; w8/wr8 are the fp8
       split of W*SCALE*WPRE; each DoubleRow contracts K=256 in 1 instr at
       0.5 cycles/row -- 4x fewer PE-cycles than the bf16 pair it replaces).
       Two consecutive relations share one PSUM bank (one 6-matmul group),
       halving PSUM->SBUF evacuation instruction count.
    3. stage-2 matmul: fp8 0/1 selection matrix reduces edge slots into node
       rows: out_psum[node, o] += SEL_r^T @ Y_bf16  (accumulated over all r;
       the evacuation folds the 1/WPRE prescale compensation).
    4. relu on PSUM->SBUF evacuation (bf16), DMA node rows to DRAM.
  Stage-1/stage-2 run as one continuous software pipeline across tiles
  (PSKEW pair-groups of skew), so a tile's stage-2 drain overlaps the next
  tile's stage-1.  The device program is fully static and identical across
  cores (SPMD); all data-dependence lives in the index / selection arrays.
  Host post-step inverse-permutes rows back to the original node order.
"""

import numpy as np
import ml_dtypes

bf16 = ml_dtypes.bfloat16
fp8 = ml_dtypes.float8_e4m3

# Problem constants (hardcoded per contract).
V = 100000
D = 256
R1 = 33          # relations incl. self-loop
N = 8192
S = 32
NCORES = 8
NPC = N // NCORES          # 1024 nodes per core
NTILES = 8                 # node-tiles per core (perfect 128-node packing)
CAPS = [128] * NTILES      # nodes per tile (uniform across cores)
ROW_BASE = np.concatenate([[0], np.cumsum(CAPS)]).tolist()
P = 128
NSLOT = R1 * P             # 4224 edge slots per tile
GSPLIT = [0, 2, 6, 18, R1]  # gather segment chunk boundaries (relations)
GSEG = [(a * P, b * P) for a, b in zip(GSPLIT, GSPLIT[1:])]
IDXW = NSLOT // 16         # 264 int16 idx columns (16-partition wrap)
UMAX = 32768               # compacted per-core embedding rows (int16 limit)
SCALE = 1000.0 / (R1 * S)
WPRE = 256.0               # fp8 W prescale; undone by sel = 2^-8
NDENSE = 3                 # head tiles loaded from a host-pre-gathered image

# Software-pipeline skew between stage-1 and stage-2 of consecutive chunks,
# so the PE never stalls on the PSUM->SBUF copy of the current chunk.
PSKEW = 4   # pipeline skew in relation-PAIRS (2 relations share a PSUM bank)
NPAIR = (R1 + 1) // 2      # 17 pair-groups per tile (last is a single)
PF = 3      # tile prefetch depth


def _q8(x):
    return np.asarray(x, np.float32).astype(fp8)


# ---------------------------------------------------------------------------
# Host-side preparation
# ---------------------------------------------------------------------------

def _balance_cores(hist, rng):
    """Assign the N nodes to NCORES cores (NPC each), minimizing the max
    per-(core, relation) edge total so every core can then be split into
    NTILES tiles with <=128-edge buckets. Greedy seed + targeted swaps."""
    order = np.argsort(-hist.max(axis=1), kind="stable")
    load = np.zeros((NCORES, R1), dtype=np.int64)
    cnt = np.zeros(NCORES, dtype=np.int64)
    coreof = np.empty(N, dtype=np.int64)
    for n in order:
        h = hist[n]
        new = load + h
        key = new.max(axis=1) * 100000 + new.sum(axis=1) // 256
        key[cnt >= NPC] = 1 << 60
        best = int(np.argmin(key))
        coreof[n] = best
        load[best] += h
        cnt[best] += 1
    target = NTILES * P - 16
    stall = 0
    for _ in range(20000):
        worst = int(load.max())
        if worst <= target or stall > 2000:
            break
        cells = np.argwhere(load >= worst)
        c, r = (int(x) for x in cells[rng.integers(len(cells))])
        cand = np.nonzero((coreof == c) & (hist[:, r] > 0))[0]
        i = int(rng.choice(cand, size=1)[0])
        hi = hist[i]
        improved = False
        for c2 in np.argsort(load[:, r])[:3]:
            c2 = int(c2)
            if c2 == c:
                continue
            cand2 = np.nonzero((coreof == c2) & (hist[:, r] == 0))[0]
            if len(cand2) == 0:
                continue
            js = rng.choice(cand2, size=min(32, len(cand2)), replace=False)
            hj = hist[js]
            n1 = (load[c] - hi)[None, :] + hj
            n2 = (load[c2] + hi)[None, :] - hj
            mx = np.maximum(n1.max(axis=1), n2.max(axis=1))
            k = int(np.argmin(mx))
            if mx[k] < worst or (mx[k] == worst
                                 and n1[k].max() < load[c].max()):
                j = int(js[k])
                load[c], load[c2] = n1[k], n2[k]
                coreof[i], coreof[j] = c2, c
                improved = True
                break
        stall = 0 if improved else stall + 1
    return coreof


def _balance_tiles(hist_c, seed=0):
    """Assign NPC nodes to NTILES tiles of exactly 128 nodes with every
    (tile, relation) bucket <= 128. Greedy seed + swap search with sideways
    moves. Returns (tiles, loads); raises if no zero-overflow split found."""
    for attempt in range(16):
        rng = np.random.default_rng(seed + attempt)
        n = len(hist_c)
        order = np.argsort(-hist_c.max(axis=1), kind="stable")
        load = np.zeros((NTILES, R1), dtype=np.int64)
        cnt = np.zeros(NTILES, dtype=np.int64)
        tileof = np.empty(n, dtype=np.int64)
        soft = P - 2
        for i in order:
            h = hist_c[i]
            new = load + h
            over = np.maximum(new - soft, 0).sum(axis=1)
            key = (over * (1 << 20) + new.max(axis=1) * 2048
                   + new.sum(axis=1) // 64)
            key[cnt >= P] = 1 << 60
            t = int(np.argmin(key))
            tileof[i] = t
            load[t] += h
            cnt[t] += 1
        cur = int(np.maximum(load - P, 0).sum())
        sideways = 0
        for _ in range(8000):
            if cur == 0:
                break
            t, r = np.unravel_index(int(np.argmax(load - P)), load.shape)
            cand = np.nonzero((tileof == t) & (hist_c[:, r] > 0))[0]
            i = int(rng.choice(cand, size=1)[0])
            hi = hist_c[i]
            best = None
            for t2 in range(NTILES):
                if t2 == t:
                    continue
                cand2 = np.nonzero((tileof == t2)
                                   & (hist_c[:, r] < hi[r]))[0]
                if len(cand2) == 0:
                    continue
                js = rng.choice(cand2, size=min(48, len(cand2)),
                                replace=False)
                hj = hist_c[js]
                n1 = (load[t] - hi)[None, :] + hj
                n2 = (load[t2] + hi)[None, :] - hj
                novr = (np.maximum(n1 - P, 0).sum(axis=1)
                        + np.maximum(n2 - P, 0).sum(axis=1)
                        + np.maximum(load - P, 0).sum()
                        - np.maximum(load[t] - P, 0).sum()
                        - np.maximum(load[t2] - P, 0).sum())
                k = int(np.argmin(novr))
                if best is None or novr[k] < best[0]:
                    best = (int(novr[k]), t2, int(js[k]), n1[k].copy(),
                            n2[k].copy())
            if best is None:
                continue
            novr_k, t2, j, n1k, n2k = best
            if novr_k < cur or (novr_k == cur and sideways < 2000
                                and rng.random() < 0.5):
                if novr_k == cur:
                    sideways += 1
                load[t], load[t2] = n1k, n2k
                tileof[i], tileof[j] = t2, t
                cur = int(np.maximum(load - P, 0).sum())
        if cur == 0:
            tiles = [np.nonzero(tileof == t)[0].tolist()
                     for t in range(NTILES)]
            return tiles, load
    raise AssertionError("tile balance failed: could not reach 0 overflow")


def prep(emb_table, weights, neighbors, relations):
    """Build per-core device arrays. Returns (in_maps, perms)."""
    emb_f = np.asarray(emb_table, dtype=np.float32)
    # W' = W*SCALE*WPRE, exact-split into fp8 w8 + wr8.
    wq = np.asarray(weights, dtype=np.float32) * (SCALE * WPRE)  # [R1, O, D]
    w8 = _q8(wq)
    wr8 = _q8(wq - w8.astype(np.float32))
    # W_sb8[p, r, which, c, o] = w{8,r8}[r, o, c*128+p]
    def wlayout(w):
        w_rdo = np.ascontiguousarray(w.transpose(0, 2, 1))      # [r, d, o]
        return w_rdo.reshape(R1, 2, 128, D).transpose(2, 0, 1, 3)  # [p,r,c,o]
    W_sb8 = np.ascontiguousarray(
        np.stack([wlayout(w8), wlayout(wr8)], axis=2)   # [p, r, which, c, o]
    )   # [128, R1, 2, 2, D] fp8

    neighbors = np.asarray(neighbors).astype(np.int64)
    relations = np.asarray(relations).astype(np.int64)

    ghist = np.zeros((N, R1), dtype=np.int64)
    np.add.at(ghist, (np.repeat(np.arange(N), S), relations.ravel()), 1)
    coreof = _balance_cores(ghist, np.random.default_rng(0))

    in_maps, perms = [], []
    for c in range(NCORES):
        cnodes = np.nonzero(coreof == c)[0]                   # global ids
        nb = neighbors[cnodes]                                # [NPC, S]
        rel = relations[cnodes]
        uniq, inv = np.unique(nb.ravel(), return_inverse=True)
        inv = inv.reshape(nb.shape).astype(np.int64)
        U = len(uniq)
        assert U <= UMAX, U
        # combined row: interleave(e8, er8) per dimension -> 512 fp8 bytes
        ef = emb_f[uniq]
        e8 = _q8(ef)
        er8 = _q8(ef - e8.astype(np.float32))
        emb_c = np.zeros((UMAX, 2 * D), dtype=fp8)
        emb_c[:U, 0::2] = e8
        emb_c[:U, 1::2] = er8

        tiles, loads = _balance_tiles(ghist[cnodes], seed=16 * c)
        assert loads.max() <= P, f"balance failed: max bucket {loads.max()}"

        idx_all = np.zeros((NTILES, 128, IDXW), dtype=np.int16)
        sel_all = np.zeros((NTILES, 128, NSLOT), dtype=fp8)
        etd_all = np.zeros((NDENSE, 128, 4 * NSLOT), dtype=fp8)
        perm = []
        for t, nodes in enumerate(tiles):
            nodes = np.array(nodes, dtype=np.int64)
            ncnt = len(nodes)
            assert ncnt == CAPS[t]
            perm.extend(cnodes[nodes].tolist())
            # edges of this tile
            er = rel[nodes].ravel()                            # relation per edge
            ei = inv[nodes].ravel()                            # compact nbr id
            ej = np.repeat(np.arange(ncnt), S)                 # local node idx
            order = np.argsort(er, kind="stable")
            er_s, ei_s, ej_s = er[order], ei[order], ej[order]
            # position within relation group
            start = np.searchsorted(er_s, np.arange(R1))
            pos = np.arange(ncnt * S) - start[er_s]
            slot = er_s * P + pos                              # [ncnt*S]
            slots_idx = np.zeros(NSLOT, dtype=np.int16)
            slots_idx[slot] = ei_s
            if t < NDENSE:
                # host-pre-gathered dense image for the pipeline head start:
                # exactly the transposed-gather SBUF layout [p, c, i, b]
                for gi, (a, b_) in enumerate(GSEG):
                    n = b_ - a
                    blk = (emb_c[slots_idx[a:b_]]
                           .reshape(n, 2, 128, 2)
                           .transpose(2, 1, 0, 3)
                           .reshape(128, 4 * n))
                    o0 = 4 * a
                    etd_all[t, :, o0:o0 + 4 * n] = blk
            sel = np.zeros((NSLOT, 128), dtype=fp8)
            sel[slot, ej_s] = fp8(1.0)
            # idx wrap per gather segment: idx i at partition i%16, col i//16
            wrapped = np.concatenate(
                [slots_idx[a:b].reshape((b - a) // 16, 16).T
                 for a, b in GSEG], axis=1)                    # [16, IDXW]
            idx_all[t] = np.tile(wrapped, (8, 1))
            # device SEL layout: [part p = slot-in-chunk, free = r*128 + node]
            sel_all[t] = np.ascontiguousarray(
                sel.reshape(R1, P, 128).transpose(1, 0, 2).reshape(P, NSLOT))
        # one partition-major idx image so the device loads it in one DMA
        idx_img = np.ascontiguousarray(
            idx_all.transpose(1, 0, 2).reshape(128, NTILES * IDXW))
        in_maps.append({
            "emb": emb_c,
            "wsb": W_sb8,
            "idx": idx_img,
            "sel": np.ascontiguousarray(sel_all.reshape(NTILES * 128, NSLOT)),
            "etd": etd_all,
        })
        perms.append(np.array(perm, dtype=np.int64))

    return in_maps, perms


# ---------------------------------------------------------------------------
# Numpy emulation (dtype-faithful) for validation
# ---------------------------------------------------------------------------

def emulate_core(in_map):
    emb = in_map["emb"]                                        # [UMAX, 512] fp8
    wsb = in_map["wsb"]                                        # [p,r,2,2,o]
    idx = in_map["idx"].reshape(128, NTILES, IDXW).transpose(1, 0, 2)
    sel = in_map["sel"].reshape(NTILES, 128, NSLOT)
    out = np.zeros((NPC, D), dtype=np.float32)
    # reconstruct w8/wr8 as [r, d, o] f32
    w = wsb.astype(np.float32)                                 # [128,r,2,2,o]
    w_rwcd = w.transpose(1, 2, 3, 0, 4)                        # [r,2,c,p,o]
    wt = w_rwcd.reshape(R1, 2, 2 * 128, D)                     # [r, which, d, o]
    e8t = emb[:, 0::2].astype(np.float32)                      # [UMAX, D]
    er8t = emb[:, 1::2].astype(np.float32)
    for t in range(NTILES):
        parts, col = [], 0
        for a, b in GSEG:
            w_ = (b - a) // 16
            parts.append(idx[t, :16, col:col + w_].T.reshape(b - a))
            col += w_
        slots_idx = np.concatenate(parts)                      # unwrap
        E8 = e8t[slots_idx]                                    # [NSLOT, D]
        ER8 = er8t[slots_idx]
        out_acc = np.zeros((128, D), dtype=np.float32)
        for r in range(R1):
            E8r = E8[r * P:(r + 1) * P]
            ER8r = ER8[r * P:(r + 1) * P]
            Y = (E8r @ wt[r, 0] + ER8r @ wt[r, 0] + E8r @ wt[r, 1])
            # evac applies the 2^-8 prescale compensation, rounds to bf16
            Yb = (Y / WPRE).astype(bf16).astype(np.float32)
            selr = sel[t][:, r * 128:(r + 1) * 128].astype(np.float32)
            out_acc += selr.T @ Yb
        base, ncnt = ROW_BASE[t], CAPS[t]
        outb = np.maximum(out_acc[:ncnt], 0.0).astype(bf16).astype(np.float32)
        out[base:base + ncnt] = outb
    return out


def emulate(emb_table, weights, neighbors, relations):
    in_maps, perms = prep(emb_table, weights, neighbors, relations)
    full = np.zeros((N, D), dtype=np.float32)
    for c in range(NCORES):
        full[perms[c]] = emulate_core(in_maps[c])
    return full


# ---------------------------------------------------------------------------
# Bass program
# ---------------------------------------------------------------------------

def build_program():
    import concourse.bacc as bacc
    import concourse.tile as tile
    import concourse.mybir as mybir

    nc = bacc.Bacc(
        "TRN2", target_bir_lowering=False, debug=False,
        num_devices=NCORES,
    )
    BF = mybir.dt.bfloat16
    F32 = mybir.dt.float32
    I16 = mybir.dt.int16
    F8 = mybir.dt.float8e4
    DR = mybir.MatmulPerfMode.DoubleRow

    emb = nc.dram_tensor("emb", [UMAX, 2 * D], F8, kind="ExternalInput").ap()
    wsb = nc.dram_tensor("wsb", [128, R1, 2, 2, D], F8,
                         kind="ExternalInput").ap()
    idx = nc.dram_tensor("idx", [128, NTILES * IDXW], I16,
                         kind="ExternalInput").ap()
    sel = nc.dram_tensor("sel", [NTILES, 128, R1, 128], F8,
                         kind="ExternalInput").ap()
    etd = nc.dram_tensor("etd", [NDENSE, 128, 4 * NSLOT], F8,
                         kind="ExternalInput").ap()
    out = nc.dram_tensor("out", [NPC, D], BF, kind="ExternalOutput").ap()

    Relu = mybir.ActivationFunctionType.Relu
    Copy = mybir.ActivationFunctionType.Copy

    with tile.TileContext(nc) as tc:
        with (
            tc.tile_pool(name="wpool", bufs=1) as wpool,
            tc.tile_pool(name="etpool", bufs=PF + 1) as etpool,
            tc.tile_pool(name="selpool", bufs=PF + 2) as selpool,
            tc.tile_pool(name="idxpool", bufs=PF + 1) as idxpool,
            tc.tile_pool(name="ypool", bufs=PSKEW + 2) as ypool,
            tc.tile_pool(name="opool", bufs=2) as opool,
            tc.tile_pool(name="psy", bufs=PSKEW + 2, space="PSUM") as psy,
            tc.tile_pool(name="pso", bufs=2, space="PSUM") as pso,
        ):
            WBATCH = [0, 2, 6, 11, 16, 21, 26, R1]
            wtiles = [
                wpool.tile([128, b - a, 2, 2, D], F8, name=f"wt{i}")
                for i, (a, b) in enumerate(zip(WBATCH, WBATCH[1:]))
            ]

            def load_w(i):
                a, b = WBATCH[i], WBATCH[i + 1]
                nc.sync.dma_start(out=wtiles[i][:], in_=wsb[:, a:b])

            def wslice(r, which):
                i = next(j for j in range(len(WBATCH) - 1)
                         if WBATCH[j] <= r < WBATCH[j + 1])
                return wtiles[i][:, r - WBATCH[i], which]

            ets, sels = {}, {}

            idxt = {}

            def pre_idx(t):
                idx_t = idxpool.tile([128, IDXW], I16, name="idx_t")
                nc.sync.dma_start(
                    out=idx_t[:], in_=idx[:, t * IDXW:(t + 1) * IDXW])
                idxt[t] = idx_t

            def pre_gather_seg(t, gi):
                a, b = GSEG[gi]
                n = b - a
                eth = etpool.tile([128, 2, n, 2], F8, name=f"et{gi}")
                if t < NDENSE:
                    # head tiles: host-pre-gathered dense image, plain DMA
                    nc.sync.dma_start(
                        out=eth[:], in_=etd[t, :, 4 * a:4 * b])
                else:
                    idx_t = idxt[t]
                    col = GSEG[gi][0] // 16
                    gview = (eth[:]
                             .rearrange("p c i b -> p c (i b)")
                             .rearrange("p c (x ii) -> p (c x) ii", x=2))
                    nc.gpsimd.dma_gather(
                        out_ap=gview,
                        in_ap=emb,
                        idxs_ap=idx_t[:, col:col + n // 16],
                        num_idxs=n,
                        num_idxs_reg=n,
                        elem_size=2 * D,
                        transpose=True,
                        single_packet=False,
                    )
                ets.setdefault(t, {})[gi] = eth

            def pre_gather(t):
                if t >= NDENSE:
                    pre_idx(t)
                for gi in range(len(GSEG)):
                    pre_gather_seg(t, gi)

            def pre_sel(t):
                sel_t = selpool.tile([128, R1, 128], F8, name="sel_t")
                nc.sync.dma_start(out=sel_t[:], in_=sel[t])
                sels[t] = sel_t

            def prefetch(t):
                if t >= NTILES:
                    return
                pre_gather(t)
                pre_sel(t)

            # startup orchestration: head tiles come from dense pre-gathered
            # images (no idx/prep/trigger latency), interleaved with W batches
            # in consumption order.
            pre_gather_seg(0, 0)
            load_w(0)
            pre_gather_seg(0, 1)
            load_w(1)
            pre_gather_seg(0, 2)
            load_w(2)
            pre_sel(0)
            pre_gather_seg(0, 3)
            load_w(3)
            load_w(4)
            load_w(5)
            load_w(6)
            pre_sel(1)
            pre_gather_seg(1, 0)
            pre_gather_seg(1, 1)
            pre_gather_seg(1, 2)
            pre_gather_seg(1, 3)
            prefetch(2)

            # continuous pipeline over (tile, pair): stage-1 of the next tile
            # overlaps the stage-2 drain of the previous one.
            total = NTILES * NPAIR
            ys = {}
            outs = {}
            cur = {}
            for k in range(total + PSKEW):
                if k < total:
                    t, m = divmod(k, NPAIR)
                    if m == 0:
                        cur["segs"] = ets.pop(t)
                        prefetch(t + PF)
                    segs = cur["segs"]
                    rels = [r for r in (2 * m, 2 * m + 1) if r < R1]
                    # one accumulation group fills both halves of a bank
                    yp = psy.tile([128, 2, D], F32, name="yp")
                    for h, r in enumerate(rels):
                        gi = next(i for i, (a, b) in enumerate(GSEG)
                                  if a <= r * P < b)
                        eth, off = segs[gi], r * P - GSEG[gi][0]
                        lA = eth[:, :, off:off + P, 0]   # e8 k-tiles
                        lB = eth[:, :, off:off + P, 1]   # er8 k-tiles
                        last = (h == len(rels) - 1)
                        w8r, wr8r = wslice(r, 0), wslice(r, 1)
                        nc.tensor.matmul(
                            out=yp[:, h], lhsT=lA, rhs=w8r,
                            start=(h == 0), stop=False, perf_mode=DR)
                        nc.tensor.matmul(
                            out=yp[:, h], lhsT=lB, rhs=w8r,
                            start=False, stop=False, perf_mode=DR)
                        nc.tensor.matmul(
                            out=yp[:, h], lhsT=lA, rhs=wr8r,
                            start=False, stop=last, perf_mode=DR)
                    ysb = ypool.tile([128, 2, D], BF, name="ysb")
                    ycopy = yp[:] if len(rels) == 2 else yp[:, 0]
                    ydst = ysb[:] if len(rels) == 2 else ysb[:, 0]
                    # evac folds the 1/WPRE prescale compensation
                    if k % 2 == 0:
                        nc.vector.tensor_scalar_mul(
                            out=ydst, in0=ycopy, scalar1=1.0 / WPRE)
                    else:
                        nc.scalar.activation(
                            out=ydst, in_=ycopy, func=Copy, scale=1.0 / WPRE)
                    ys[k] = ysb
                if k >= PSKEW:
                    t2, q = divmod(k - PSKEW, NPAIR)
                    if q == 0:
                        outs[t2] = pso.tile([128, D], F32, name="outp")
                    outp = outs[t2]
                    sel_t2 = sels[t2]
                    ysb_q = ys.pop(k - PSKEW)
                    for h, r in enumerate(
                            [r for r in (2 * q, 2 * q + 1) if r < R1]):
                        nc.tensor.matmul(
                            out=outp[:],
                            lhsT=sel_t2[:, r],
                            rhs=ysb_q[:, h],
                            start=(r == 0), stop=(r == R1 - 1),
                        )
                    if q == NPAIR - 1:
                        outs.pop(t2)
                        sels.pop(t2)
                        osb = opool.tile([128, D], BF)
                        nc.scalar.activation(out=osb[:], in_=outp[:],
                                             func=Relu)
                        base, ncnt = ROW_BASE[t2], CAPS[t2]
                        nc.sync.dma_start(
                            out=out[base:base + ncnt, :], in_=osb[:ncnt, :])

    nc.compile()
    return nc


_NC_CACHE = []


def _get_program():
    if not _NC_CACHE:
        _NC_CACHE.append(build_program())
    return _NC_CACHE[0]


# ---------------------------------------------------------------------------
# Entry point
# ---------------------------------------------------------------------------

def kernel(emb_table, weights, neighbors, relations):
    from concourse import bass_utils

    in_maps, perms = prep(emb_table, weights, neighbors, relations)
    nc = _get_program()
    res = bass_utils.run_bass_kernel_spmd(
        nc, in_maps, core_ids=list(range(NCORES)),
    )
    full = np.zeros((N, D), dtype=np.float32)
    for c in range(NCORES):
        full[perms[c]] = res.results[c]["out"].astype(np.float32)
    return full


# revision 136
# speedup vs baseline: 1.0038x; 1.0038x over previous
"""Trainium2 Bass kernel for LoopRelationalGraphConvolution.

Math (matches the jax reference):
    out[n] = relu( SCALE * sum_s  W[rel[n,s]] @ emb[neighbors[n,s]] )
    SCALE  = 1000 / (R1 * S)      (folds the mean over S and the /R1 * 1000)

Design (8 NeuronCores, data-parallel over the 8192-node batch):
  A two-level host balancer assigns nodes to cores and then to 8 tiles of
  exactly 128 nodes per core such that every (tile, relation) bucket has
  <=128 edges (zero overflow).  Per tile the device kernel:
    1. dma_gather(transpose=True): fetches the tile's 33*128 edge-slot
       embeddings from a combined fp8 table whose 512-byte rows interleave
       e8 = fp8(emb) and er8 = fp8(emb - e8) per dimension, so the 16-bit
       transpose granularity lands  ET[p, c, i, b] = (e8|er8)[idx_i][c*128+p].
       The first NDENSE tiles instead load host-pre-gathered dense images
       with plain DMAs, eliminating the idx->prep->trigger latency chain at
       pipeline startup (which is otherwise serialized behind the W load).
    2. stage-1: per relation r, THREE fp8 DoubleRow matmuls accumulate
       Y[slot, o] = e8*w8 + er8*w8 + e8*wr8   (PSUM f32; w8/wr8 are the fp8
       split of W*SCALE*WPRE; each DoubleRow contracts K=256 in 1 instr at
       0.5 cycles/row -- 4x fewer PE-cycles than the bf16 pair it replaces).
       Two consecutive relations share one PSUM bank (one 6-matmul group),
       halving PSUM->SBUF evacuation instruction count.
    3. stage-2 matmul: fp8 0/1 selection matrix reduces edge slots into node
       rows: out_psum[node, o] += SEL_r^T @ Y_bf16  (accumulated over all r;
       the evacuation folds the 1/WPRE prescale compensation).
    4. relu on PSUM->SBUF evacuation (bf16), DMA node rows to DRAM.
  Stage-1/stage-2 run as one continuous software pipeline across tiles
  (PSKEW pair-groups of skew), so a tile's stage-2 drain overlaps the next
  tile's stage-1.  The device program is fully static and identical across
  cores (SPMD); all data-dependence lives in the index / selection arrays.
  Host post-step inverse-permutes rows back to the original node order.
"""

import numpy as np
import ml_dtypes

bf16 = ml_dtypes.bfloat16
fp8 = ml_dtypes.float8_e4m3

# Problem constants (hardcoded per contract).
V = 100000
D = 256
R1 = 33          # relations incl. self-loop
N = 8192
S = 32
NCORES = 8
NPC = N // NCORES          # 1024 nodes per core
NTILES = 8                 # node-tiles per core (perfect 128-node packing)
CAPS = [128] * NTILES      # nodes per tile (uniform across cores)
ROW_BASE = np.concatenate([[0], np.cumsum(CAPS)]).tolist()
P = 128
NSLOT = R1 * P             # 4224 edge slots per tile
GSPLIT = [0, 2, 6, 18, R1]  # gather segment chunk boundaries (relations)
GSEG = [(a * P, b * P) for a, b in zip(GSPLIT, GSPLIT[1:])]
IDXW = NSLOT // 16         # 264 int16 idx columns (16-partition wrap)
UMAX = 32768               # compacted per-core embedding rows (int16 limit)
SCALE = 1000.0 / (R1 * S)
WPRE = 256.0               # fp8 W prescale; undone by sel = 2^-8
NDENSE = 3                 # head tiles loaded from a host-pre-gathered image

# Software-pipeline skew between stage-1 and stage-2 of consecutive chunks,
# so the PE never stalls on the PSUM->SBUF copy of the current chunk.
PSKEW = 4   # pipeline skew in relation-PAIRS (2 relations share a PSUM bank)
NPAIR = (R1 + 1) // 2      # 17 pair-groups per tile (last is a single)
PF = 3      # tile prefetch depth


def _q8(x):
    return np.asarray(x, np.float32).astype(fp8)


# ---------------------------------------------------------------------------
# Host-side preparation
# ---------------------------------------------------------------------------

def _balance_cores(hist, rng):
    """Assign the N nodes to NCORES cores (NPC each), minimizing the max
    per-(core, relation) edge total so every core can then be split into
    NTILES tiles with <=128-edge buckets. Greedy seed + targeted swaps."""
    order = np.argsort(-hist.max(axis=1), kind="stable")
    load = np.zeros((NCORES, R1), dtype=np.int64)
    cnt = np.zeros(NCORES, dtype=np.int64)
    coreof = np.empty(N, dtype=np.int64)
    for n in order:
        h = hist[n]
        new = load + h
        key = new.max(axis=1) * 100000 + new.sum(axis=1) // 256
        key[cnt >= NPC] = 1 << 60
        best = int(np.argmin(key))
        coreof[n] = best
        load[best] += h
        cnt[best] += 1
    target = NTILES * P - 16
    stall = 0
    for _ in range(20000):
        worst = int(load.max())
        if worst <= target or stall > 2000:
            break
        cells = np.argwhere(load >= worst)
        c, r = (int(x) for x in cells[rng.integers(len(cells))])
        cand = np.nonzero((coreof == c) & (hist[:, r] > 0))[0]
        i = int(rng.choice(cand, size=1)[0])
        hi = hist[i]
        improved = False
        for c2 in np.argsort(load[:, r])[:3]:
            c2 = int(c2)
            if c2 == c:
                continue
            cand2 = np.nonzero((coreof == c2) & (hist[:, r] == 0))[0]
            if len(cand2) == 0:
                continue
            js = rng.choice(cand2, size=min(32, len(cand2)), replace=False)
            hj = hist[js]
            n1 = (load[c] - hi)[None, :] + hj
            n2 = (load[c2] + hi)[None, :] - hj
            mx = np.maximum(n1.max(axis=1), n2.max(axis=1))
            k = int(np.argmin(mx))
            if mx[k] < worst or (mx[k] == worst
                                 and n1[k].max() < load[c].max()):
                j = int(js[k])
                load[c], load[c2] = n1[k], n2[k]
                coreof[i], coreof[j] = c2, c
                improved = True
                break
        stall = 0 if improved else stall + 1
    return coreof


def _balance_tiles(hist_c, seed=0):
    """Assign NPC nodes to NTILES tiles of exactly 128 nodes with every
    (tile, relation) bucket <= 128. Greedy seed + swap search with sideways
    moves. Returns (tiles, loads); raises if no zero-overflow split found."""
    for attempt in range(16):
        rng = np.random.default_rng(seed + attempt)
        n = len(hist_c)
        order = np.argsort(-hist_c.max(axis=1), kind="stable")
        load = np.zeros((NTILES, R1), dtype=np.int64)
        cnt = np.zeros(NTILES, dtype=np.int64)
        tileof = np.empty(n, dtype=np.int64)
        soft = P - 2
        for i in order:
            h = hist_c[i]
            new = load + h
            over = np.maximum(new - soft, 0).sum(axis=1)
            key = (over * (1 << 20) + new.max(axis=1) * 2048
                   + new.sum(axis=1) // 64)
            key[cnt >= P] = 1 << 60
            t = int(np.argmin(key))
            tileof[i] = t
            load[t] += h
            cnt[t] += 1
        cur = int(np.maximum(load - P, 0).sum())
        sideways = 0
        for _ in range(8000):
            if cur == 0:
                break
            t, r = np.unravel_index(int(np.argmax(load - P)), load.shape)
            cand = np.nonzero((tileof == t) & (hist_c[:, r] > 0))[0]
            i = int(rng.choice(cand, size=1)[0])
            hi = hist_c[i]
            best = None
            for t2 in range(NTILES):
                if t2 == t:
                    continue
                cand2 = np.nonzero((tileof == t2)
                                   & (hist_c[:, r] < hi[r]))[0]
                if len(cand2) == 0:
                    continue
                js = rng.choice(cand2, size=min(48, len(cand2)),
                                replace=False)
                hj = hist_c[js]
                n1 = (load[t] - hi)[None, :] + hj
                n2 = (load[t2] + hi)[None, :] - hj
                novr = (np.maximum(n1 - P, 0).sum(axis=1)
                        + np.maximum(n2 - P, 0).sum(axis=1)
                        + np.maximum(load - P, 0).sum()
                        - np.maximum(load[t] - P, 0).sum()
                        - np.maximum(load[t2] - P, 0).sum())
                k = int(np.argmin(novr))
                if best is None or novr[k] < best[0]:
                    best = (int(novr[k]), t2, int(js[k]), n1[k].copy(),
                            n2[k].copy())
            if best is None:
                continue
            novr_k, t2, j, n1k, n2k = best
            if novr_k < cur or (novr_k == cur and sideways < 2000
                                and rng.random() < 0.5):
                if novr_k == cur:
                    sideways += 1
                load[t], load[t2] = n1k, n2k
                tileof[i], tileof[j] = t2, t
                cur = int(np.maximum(load - P, 0).sum())
        if cur == 0:
            tiles = [np.nonzero(tileof == t)[0].tolist()
                     for t in range(NTILES)]
            return tiles, load
    raise AssertionError("tile balance failed: could not reach 0 overflow")


def prep(emb_table, weights, neighbors, relations):
    """Build per-core device arrays. Returns (in_maps, perms)."""
    emb_f = np.asarray(emb_table, dtype=np.float32)
    # W' = W*SCALE*WPRE, exact-split into fp8 w8 + wr8.
    wq = np.asarray(weights, dtype=np.float32) * (SCALE * WPRE)  # [R1, O, D]
    w8 = _q8(wq)
    wr8 = _q8(wq - w8.astype(np.float32))
    # W_sb8[p, r, which, c, o] = w{8,r8}[r, o, c*128+p]
    def wlayout(w):
        w_rdo = np.ascontiguousarray(w.transpose(0, 2, 1))      # [r, d, o]
        return w_rdo.reshape(R1, 2, 128, D).transpose(2, 0, 1, 3)  # [p,r,c,o]
    W_sb8 = np.ascontiguousarray(
        np.stack([wlayout(w8), wlayout(wr8)], axis=2)   # [p, r, which, c, o]
    )   # [128, R1, 2, 2, D] fp8

    neighbors = np.asarray(neighbors).astype(np.int64)
    relations = np.asarray(relations).astype(np.int64)

    ghist = np.zeros((N, R1), dtype=np.int64)
    np.add.at(ghist, (np.repeat(np.arange(N), S), relations.ravel()), 1)
    coreof = _balance_cores(ghist, np.random.default_rng(0))

    in_maps, perms = [], []
    for c in range(NCORES):
        cnodes = np.nonzero(coreof == c)[0]                   # global ids
        nb = neighbors[cnodes]                                # [NPC, S]
        rel = relations[cnodes]
        uniq, inv = np.unique(nb.ravel(), return_inverse=True)
        inv = inv.reshape(nb.shape).astype(np.int64)
        U = len(uniq)
        assert U <= UMAX, U
        # combined row: interleave(e8, er8) per dimension -> 512 fp8 bytes
        ef = emb_f[uniq]
        e8 = _q8(ef)
        er8 = _q8(ef - e8.astype(np.float32))
        emb_c = np.zeros((UMAX, 2 * D), dtype=fp8)
        emb_c[:U, 0::2] = e8
        emb_c[:U, 1::2] = er8

        tiles, loads = _balance_tiles(ghist[cnodes], seed=16 * c)
        assert loads.max() <= P, f"balance failed: max bucket {loads.max()}"

        idx_all = np.zeros((NTILES, 128, IDXW), dtype=np.int16)
        sel_all = np.zeros((NTILES, 128, NSLOT), dtype=fp8)
        etd_all = np.zeros((NDENSE, 128, 4 * NSLOT), dtype=fp8)
        perm = []
        for t, nodes in enumerate(tiles):
            nodes = np.array(nodes, dtype=np.int64)
            ncnt = len(nodes)
            assert ncnt == CAPS[t]
            perm.extend(cnodes[nodes].tolist())
            # edges of this tile
            er = rel[nodes].ravel()                            # relation per edge
            ei = inv[nodes].ravel()                            # compact nbr id
            ej = np.repeat(np.arange(ncnt), S)                 # local node idx
            order = np.argsort(er, kind="stable")
            er_s, ei_s, ej_s = er[order], ei[order], ej[order]
            # position within relation group
            start = np.searchsorted(er_s, np.arange(R1))
            pos = np.arange(ncnt * S) - start[er_s]
            slot = er_s * P + pos                              # [ncnt*S]
            slots_idx = np.zeros(NSLOT, dtype=np.int16)
            slots_idx[slot] = ei_s
            if t < NDENSE:
                # host-pre-gathered dense image for the pipeline head start:
                # exactly the transposed-gather SBUF layout [p, c, i, b]
                for gi, (a, b_) in enumerate(GSEG):
                    n = b_ - a
                    blk = (emb_c[slots_idx[a:b_]]
                           .reshape(n, 2, 128, 2)
                           .transpose(2, 1, 0, 3)
                           .reshape(128, 4 * n))
                    o0 = 4 * a
                    etd_all[t, :, o0:o0 + 4 * n] = blk
            sel = np.zeros((NSLOT, 128), dtype=fp8)
            sel[slot, ej_s] = fp8(1.0)
            # idx wrap per gather segment: idx i at partition i%16, col i//16
            wrapped = np.concatenate(
                [slots_idx[a:b].reshape((b - a) // 16, 16).T
                 for a, b in GSEG], axis=1)                    # [16, IDXW]
            idx_all[t] = np.tile(wrapped, (8, 1))
            # device SEL layout: [part p = slot-in-chunk, free = r*128 + node]
            sel_all[t] = np.ascontiguousarray(
                sel.reshape(R1, P, 128).transpose(1, 0, 2).reshape(P, NSLOT))
        # one partition-major idx image so the device loads it in one DMA
        idx_img = np.ascontiguousarray(
            idx_all.transpose(1, 0, 2).reshape(128, NTILES * IDXW))
        in_maps.append({
            "emb": emb_c,
            "wsb": W_sb8,
            "idx": idx_img,
            "sel": np.ascontiguousarray(sel_all.reshape(NTILES * 128, NSLOT)),
            "etd": etd_all,
        })
        perms.append(np.array(perm, dtype=np.int64))

    return in_maps, perms


# ---------------------------------------------------------------------------
# Numpy emulation (dtype-faithful) for validation
# ---------------------------------------------------------------------------

def emulate_core(in_map):
    emb = in_map["emb"]                                        # [UMAX, 512] fp8
    wsb = in_map["wsb"]                                        # [p,r,2,2,o]
    idx = in_map["idx"].reshape(128, NTILES, IDXW).transpose(1, 0, 2)
    sel = in_map["sel"].reshape(NTILES, 128, NSLOT)
    out = np.zeros((NPC, D), dtype=np.float32)
    # reconstruct w8/wr8 as [r, d, o] f32
    w = wsb.astype(np.float32)                                 # [128,r,2,2,o]
    w_rwcd = w.transpose(1, 2, 3, 0, 4)                        # [r,2,c,p,o]
    wt = w_rwcd.reshape(R1, 2, 2 * 128, D)                     # [r, which, d, o]
    e8t = emb[:, 0::2].astype(np.float32)                      # [UMAX, D]
    er8t = emb[:, 1::2].astype(np.float32)
    for t in range(NTILES):
        parts, col = [], 0
        for a, b in GSEG:
            w_ = (b - a) // 16
            parts.append(idx[t, :16, col:col + w_].T.reshape(b - a))
            col += w_
        slots_idx = np.concatenate(parts)                      # unwrap
        E8 = e8t[slots_idx]                                    # [NSLOT, D]
        ER8 = er8t[slots_idx]
        out_acc = np.zeros((128, D), dtype=np.float32)
        for r in range(R1):
            E8r = E8[r * P:(r + 1) * P]
            ER8r = ER8[r * P:(r + 1) * P]
            Y = (E8r @ wt[r, 0] + ER8r @ wt[r, 0] + E8r @ wt[r, 1])
            # evac applies the 2^-8 prescale compensation, rounds to bf16
            Yb = (Y / WPRE).astype(bf16).astype(np.float32)
            selr = sel[t][:, r * 128:(r + 1) * 128].astype(np.float32)
            out_acc += selr.T @ Yb
        base, ncnt = ROW_BASE[t], CAPS[t]
        outb = np.maximum(out_acc[:ncnt], 0.0).astype(bf16).astype(np.float32)
        out[base:base + ncnt] = outb
    return out


def emulate(emb_table, weights, neighbors, relations):
    in_maps, perms = prep(emb_table, weights, neighbors, relations)
    full = np.zeros((N, D), dtype=np.float32)
    for c in range(NCORES):
        full[perms[c]] = emulate_core(in_maps[c])
    return full


# ---------------------------------------------------------------------------
# Bass program
# ---------------------------------------------------------------------------

def build_program():
    import concourse.bacc as bacc
    import concourse.tile as tile
    import concourse.mybir as mybir

    nc = bacc.Bacc(
        "TRN2", target_bir_lowering=False, debug=False,
        num_devices=NCORES,
    )
    BF = mybir.dt.bfloat16
    F32 = mybir.dt.float32
    I16 = mybir.dt.int16
    F8 = mybir.dt.float8e4
    DR = mybir.MatmulPerfMode.DoubleRow

    emb = nc.dram_tensor("emb", [UMAX, 2 * D], F8, kind="ExternalInput").ap()
    wsb = nc.dram_tensor("wsb", [128, R1, 2, 2, D], F8,
                         kind="ExternalInput").ap()
    idx = nc.dram_tensor("idx", [128, NTILES * IDXW], I16,
                         kind="ExternalInput").ap()
    sel = nc.dram_tensor("sel", [NTILES, 128, R1, 128], F8,
                         kind="ExternalInput").ap()
    etd = nc.dram_tensor("etd", [NDENSE, 128, 4 * NSLOT], F8,
                         kind="ExternalInput").ap()
    out = nc.dram_tensor("out", [NPC, D], BF, kind="ExternalOutput").ap()

    Relu = mybir.ActivationFunctionType.Relu
    Copy = mybir.ActivationFunctionType.Copy

    with tile.TileContext(nc) as tc:
        with (
            tc.tile_pool(name="wpool", bufs=1) as wpool,
            tc.tile_pool(name="etpool", bufs=PF + 1) as etpool,
            tc.tile_pool(name="selpool", bufs=PF + 2) as selpool,
            tc.tile_pool(name="idxpool", bufs=PF + 1) as idxpool,
            tc.tile_pool(name="ypool", bufs=PSKEW + 2) as ypool,
            tc.tile_pool(name="opool", bufs=2) as opool,
            tc.tile_pool(name="psy", bufs=PSKEW + 2, space="PSUM") as psy,
            tc.tile_pool(name="pso", bufs=2, space="PSUM") as pso,
        ):
            WBATCH = [0, 2, 6, 11, 16, 21, 26, R1]
            wtiles = [
                wpool.tile([128, b - a, 2, 2, D], F8, name=f"wt{i}")
                for i, (a, b) in enumerate(zip(WBATCH, WBATCH[1:]))
            ]

            def load_w(i):
                a, b = WBATCH[i], WBATCH[i + 1]
                nc.sync.dma_start(out=wtiles[i][:], in_=wsb[:, a:b])

            def wslice(r, which):
                i = next(j for j in range(len(WBATCH) - 1)
                         if WBATCH[j] <= r < WBATCH[j + 1])
                return wtiles[i][:, r - WBATCH[i], which]

            ets, sels = {}, {}

            idxt = {}

            def pre_idx(t):
                idx_t = idxpool.tile([128, IDXW], I16, name="idx_t")
                nc.sync.dma_start(
                    out=idx_t[:], in_=idx[:, t * IDXW:(t + 1) * IDXW])
                idxt[t] = idx_t

            def pre_gather_seg(t, gi):
                a, b = GSEG[gi]
                n = b - a
                eth = etpool.tile([128, 2, n, 2], F8, name=f"et{gi}")
                if t < NDENSE:
                    # head tiles: host-pre-gathered dense image, plain DMA
                    nc.sync.dma_start(
                        out=eth[:], in_=etd[t, :, 4 * a:4 * b])
                else:
                    idx_t = idxt[t]
                    col = GSEG[gi][0] // 16
                    gview = (eth[:]
                             .rearrange("p c i b -> p c (i b)")
                             .rearrange("p c (x ii) -> p (c x) ii", x=2))
                    nc.gpsimd.dma_gather(
                        out_ap=gview,
                        in_ap=emb,
                        idxs_ap=idx_t[:, col:col + n // 16],
                        num_idxs=n,
                        num_idxs_reg=n,
                        elem_size=2 * D,
                        transpose=True,
                        single_packet=False,
                    )
                ets.setdefault(t, {})[gi] = eth

            def pre_gather(t):
                if t >= NDENSE:
                    pre_idx(t)
                for gi in range(len(GSEG)):
                    pre_gather_seg(t, gi)

            def pre_sel(t):
                sel_t = selpool.tile([128, R1, 128], F8, name="sel_t")
                eng = nc.gpsimd if t < NDENSE else nc.sync
                eng.dma_start(out=sel_t[:], in_=sel[t])
                sels[t] = sel_t

            def prefetch(t):
                if t >= NTILES:
                    return
                pre_gather(t)
                pre_sel(t)

            # startup orchestration: head tiles come from dense pre-gathered
            # images (no idx/prep/trigger latency), interleaved with W batches
            # in consumption order.
            pre_gather_seg(0, 0)
            load_w(0)
            pre_gather_seg(0, 1)
            load_w(1)
            pre_gather_seg(0, 2)
            load_w(2)
            pre_sel(0)
            pre_gather_seg(0, 3)
            load_w(3)
            load_w(4)
            load_w(5)
            load_w(6)
            pre_sel(1)
            pre_gather_seg(1, 0)
            pre_gather_seg(1, 1)
            pre_gather_seg(1, 2)
            pre_gather_seg(1, 3)
            prefetch(2)

            # continuous pipeline over (tile, pair): stage-1 of the next tile
            # overlaps the stage-2 drain of the previous one.
            total = NTILES * NPAIR
            ys = {}
            outs = {}
            cur = {}
            for k in range(total + PSKEW):
                if k < total:
                    t, m = divmod(k, NPAIR)
                    if m == 0:
                        cur["segs"] = ets.pop(t)
                        prefetch(t + PF)
                    segs = cur["segs"]
                    rels = [r for r in (2 * m, 2 * m + 1) if r < R1]
                    # one accumulation group fills both halves of a bank
                    yp = psy.tile([128, 2, D], F32, name="yp")
                    for h, r in enumerate(rels):
                        gi = next(i for i, (a, b) in enumerate(GSEG)
                                  if a <= r * P < b)
                        eth, off = segs[gi], r * P - GSEG[gi][0]
                        lA = eth[:, :, off:off + P, 0]   # e8 k-tiles
                        lB = eth[:, :, off:off + P, 1]   # er8 k-tiles
                        last = (h == len(rels) - 1)
                        w8r, wr8r = wslice(r, 0), wslice(r, 1)
                        nc.tensor.matmul(
                            out=yp[:, h], lhsT=lA, rhs=w8r,
                            start=(h == 0), stop=False, perf_mode=DR)
                        nc.tensor.matmul(
                            out=yp[:, h], lhsT=lB, rhs=w8r,
                            start=False, stop=False, perf_mode=DR)
                        nc.tensor.matmul(
                            out=yp[:, h], lhsT=lA, rhs=wr8r,
                            start=False, stop=last, perf_mode=DR)
                    ysb = ypool.tile([128, 2, D], BF, name="ysb")
                    ycopy = yp[:] if len(rels) == 2 else yp[:, 0]
                    ydst = ysb[:] if len(rels) == 2 else ysb[:, 0]
                    # evac folds the 1/WPRE prescale compensation
                    if k % 2 == 0:
                        nc.vector.tensor_scalar_mul(
                            out=ydst, in0=ycopy, scalar1=1.0 / WPRE)
                    else:
                        nc.scalar.activation(
                            out=ydst, in_=ycopy, func=Copy, scale=1.0 / WPRE)
                    ys[k] = ysb
                if k >= PSKEW:
                    t2, q = divmod(k - PSKEW, NPAIR)
                    if q == 0:
                        outs[t2] = pso.tile([128, D], F32, name="outp")
                    outp = outs[t2]
                    sel_t2 = sels[t2]
                    ysb_q = ys.pop(k - PSKEW)
                    for h, r in enumerate(
                            [r for r in (2 * q, 2 * q + 1) if r < R1]):
                        nc.tensor.matmul(
                            out=outp[:],
                            lhsT=sel_t2[:, r],
                            rhs=ysb_q[:, h],
                            start=(r == 0), stop=(r == R1 - 1),
                        )
                    if q == NPAIR - 1:
                        outs.pop(t2)
                        sels.pop(t2)
                        osb = opool.tile([128, D], BF)
                        nc.scalar.activation(out=osb[:], in_=outp[:],
                                             func=Relu)
                        base, ncnt = ROW_BASE[t2], CAPS[t2]
                        nc.gpsimd.dma_start(
                            out=out[base:base + ncnt, :], in_=osb[:ncnt, :])

    nc.compile()
    return nc


_NC_CACHE = []


def _get_program():
    if not _NC_CACHE:
        _NC_CACHE.append(build_program())
    return _NC_CACHE[0]


# ---------------------------------------------------------------------------
# Entry point
# ---------------------------------------------------------------------------

def kernel(emb_table, weights, neighbors, relations):
    from concourse import bass_utils

    in_maps, perms = prep(emb_table, weights, neighbors, relations)
    nc = _get_program()
    res = bass_utils.run_bass_kernel_spmd(
        nc, in_maps, core_ids=list(range(NCORES)),
    )
    full = np.zeros((N, D), dtype=np.float32)
    for c in range(NCORES):
        full[perms[c]] = res.results[c]["out"].astype(np.float32)
    return full
